# revision 9
# baseline (speedup 1.0000x reference)
"""Trainium2 Bass kernel for nn_Attention (B=4, T=1024, C=1024, 16 heads).

Sharding: 8 cores = (batch b, query-half q). Core i handles queries
t in [q*512, q*512+512) of batch b = i//2, computing K/V for the whole
batch locally (no collectives). Host gather is a pure concatenation.

Everything on-chip is laid out to avoid transposes:
  - host passes x^T (tokens rotated so the query half is first; key
    order is a permutation, which softmax-attention is invariant to)
  - Q^T, K^T come out of the projection in [o, t] layout directly
  - V is produced in [t, o] layout with a ones-column appended per head
    so the PV matmul also yields the softmax denominator Z for free
  - softmax runs on S^T = (QK^T)^T, normalization multiplies by a
    PE-broadcast reciprocal row, and the output projection consumes
    out^T [c2, t] directly, yielding y^T which the host transposes back.

All matmuls are float32r (TF32-like 4-byte fast path, 1 cycle/row for
N=512 moving operands) with fp32 PSUM accumulation.
"""

import numpy as np

B, T, C = 4, 1024, 1024
NH, HD = 16, 64
TQ = T // 2
KC = C // 128  # 8 contraction chunks
SCALE = 1.0 / float(np.sqrt(HD))

_PROG = None


def _build_program():
    import concourse.bacc as bacc
    import concourse.mybir as mybir
    import concourse.tile as tile

    F32 = mybir.dt.float32
    F32R = mybir.dt.float32r
    Exp = mybir.ActivationFunctionType.Exp
    Ident = mybir.ActivationFunctionType.Identity

    def r(ap):
        return ap.bitcast(F32R)

    nc = bacc.Bacc()
    xt_d = nc.declare_dram_parameter("xt", [KC, 128, T], F32R, isOutput=False)
    wq_d = nc.declare_dram_parameter("wq", [KC, 8, 128, 128], F32R, isOutput=False)
    wk_d = nc.declare_dram_parameter("wk", [KC, 8, 128, 128], F32R, isOutput=False)
    wv_d = nc.declare_dram_parameter("wv", [KC, 128, C], F32R, isOutput=False)
    wp_d = nc.declare_dram_parameter("wp", [KC, 8, 128, 128], F32R, isOutput=False)
    bias_d = nc.declare_dram_parameter("bias", [8, 128, 1], F32, isOutput=False)
    ones_d = nc.declare_dram_parameter("ones", [128, 128], F32R, isOutput=False)
    yt_d = nc.declare_dram_parameter("yt", [8, 128, TQ], F32, isOutput=True)

    with tile.TileContext(nc) as tc, nc.allow_low_precision(
        "float32r matmul inputs (TF32-like) are intentional"
    ):
        with (
            tc.tile_pool(name="xt", bufs=KC) as xt_pool,
            tc.tile_pool(name="wstat", bufs=3) as wstat_pool,
            tc.tile_pool(name="wmov", bufs=KC) as wmov_pool,
            tc.tile_pool(name="kt", bufs=KC) as kt_pool,
            tc.tile_pool(name="qt", bufs=KC) as qt_pool,
            tc.tile_pool(name="vaug", bufs=KC) as v_pool,
            tc.tile_pool(name="exp", bufs=8) as exp_pool,
            tc.tile_pool(name="ot", bufs=KC) as ot_pool,
            tc.tile_pool(name="small", bufs=1) as small_pool,
            tc.tile_pool(name="ysb", bufs=2) as y_pool,
            tc.tile_pool(name="osb", bufs=2) as osb_pool,
            tc.tile_pool(name="osb2", bufs=2) as osb2_pool,
            tc.tile_pool(name="rsb", bufs=2) as r_pool,
            tc.tile_pool(name="bias", bufs=8) as bias_pool,
            tc.tile_pool(name="ps", bufs=8, space="PSUM") as ps_pool,
        ):
            # ---- stage 0: loads and constants -------------------------
            xt = []
            for k in range(KC):
                t_ = xt_pool.tile([128, T], F32R, tag="xt")
                nc.sync.dma_start(t_[:], xt_d[k])
                xt.append(t_)

            ones65 = small_pool.tile([65, 128], F32R, tag="ones")
            nc.sync.dma_start(ones65[64:65, :], ones_d[0:1, :])

            bias_sb = []
            for m in range(8):
                bt = bias_pool.tile([128, 1], F32, tag="bias")
                nc.sync.dma_start(bt[:], bias_d[m])
                bias_sb.append(bt)

            va = []
            for m in range(KC):
                vt = v_pool.tile([128, NH * 65], F32R, tag="vaug")
                view = vt[:].rearrange("p (h e) -> p h e", e=65)
                nc.sync.dma_start(view[:, :, 64:65], ones_d[:, 0:NH])
                va.append(vt)

            # ---- stage 1a: Q^T = Wq @ x^T  ([o,tq], query half only) --
            qt = []
            for ko in range(KC):
                qw = wstat_pool.tile([128, C], F32R, tag="wstat")
                for k in range(KC):
                    nc.sync.dma_start(qw[:, k * 128:(k + 1) * 128], wq_d[k, ko])
                ps = ps_pool.tile([128, TQ], F32, tag="ps")
                for k in range(KC):
                    nc.tensor.matmul(
                        ps[:], r(qw[:, k * 128:(k + 1) * 128]), r(xt[k][:, 0:TQ]),
                        start=(k == 0), stop=(k == KC - 1),
                    )
                q_ = qt_pool.tile([128, TQ], F32R, tag="qt")
                nc.vector.tensor_copy(q_[:], ps[:])
                qt.append(q_)

            # ---- stage 1b: K^T = Wk @ x^T  ([o,t], all tokens) --------
            kt = []
            for ko in range(KC):
                kw = wstat_pool.tile([128, C], F32R, tag="wstat")
                for k in range(KC):
                    nc.sync.dma_start(kw[:, k * 128:(k + 1) * 128], wk_d[k, ko])
                k_ = kt_pool.tile([128, T], F32R, tag="kt")
                for n in range(2):
                    ps = ps_pool.tile([128, 512], F32, tag="ps")
                    for k in range(KC):
                        nc.tensor.matmul(
                            ps[:], r(kw[:, k * 128:(k + 1) * 128]),
                            r(xt[k][:, n * 512:(n + 1) * 512]),
                            start=(k == 0), stop=(k == KC - 1),
                        )
                    nc.vector.tensor_copy(k_[:, n * 512:(n + 1) * 512], ps[:])
                kt.append(k_)

            # ---- stage 1c: V = x @ Wv^T  ([t,o] + ones columns) -------
            wv_sb = []
            for k in range(KC):
                wvt = wmov_pool.tile([128, C], F32R, tag="wmov")
                nc.sync.dma_start(wvt[:], wv_d[k])
                wv_sb.append(wvt)
            for m in range(KC):
                view = va[m][:].rearrange("p (h e) -> p h e", e=65)
                for n in range(2):
                    ps = ps_pool.tile([128, 512], F32, tag="ps")
                    for k in range(KC):
                        nc.tensor.matmul(
                            ps[:], r(xt[k][:, m * 128:(m + 1) * 128]),
                            r(wv_sb[k][:, n * 512:(n + 1) * 512]),
                            start=(k == 0), stop=(k == KC - 1),
                        )
                    src = ps[:].rearrange("p (h d) -> p h d", d=64)
                    nc.vector.tensor_copy(view[:, n * 8:(n + 1) * 8, 0:64], src)

            # ---- stage 2: per-head SDPA (no-max softmax) --------------
            ot = []
            for k in range(KC):
                o_ = ot_pool.tile([128, TQ], F32R, tag="ot")
                ot.append(o_)

            def emit_norm(h, ops):
                kc, half = divmod(h, 2)
                po = half * 64
                rt = r_pool.tile([65, TQ], F32R, tag="rsb")
                nc.vector.reciprocal(rt[64:65, :], ops[64:65, :])
                rb = ps_pool.tile([128, TQ], F32, tag="ps")
                nc.tensor.matmul(
                    rb[:], r(ones65[64:65, :]), r(rt[64:65, :]),
                    start=True, stop=True,
                )
                ob = osb_pool.tile([64, TQ], F32, tag="osb")
                nc.vector.tensor_copy(ob[:], ops[0:64, :])
                ob2 = osb2_pool.tile([64, TQ], F32R, tag="osb2")
                nc.vector.tensor_mul(ob2[:], ob[:], rb[0:64, :])
                nc.sync.dma_start(ot[kc][po:po + 64, :], ob2[:])

            pend = None
            for h in range(NH):
                kc, half = divmod(h, 2)
                po = half * 64
                exps = []
                for j in range(KC):
                    sps = ps_pool.tile([128, TQ], F32, tag="ps")
                    nc.tensor.matmul(
                        sps[:],
                        r(kt[kc][po:po + 64, j * 128:(j + 1) * 128]),
                        r(qt[kc][po:po + 64, :]),
                        start=True, stop=True,
                    )
                    e = exp_pool.tile([128, TQ], F32R, tag="exp")
                    nc.scalar.activation(e[:], sps[:], Exp, scale=SCALE)
                    exps.append(e)
                if pend is not None:
                    emit_norm(*pend)
                ops = ps_pool.tile([65, TQ], F32, tag="ps")
                for j in range(KC):
                    nc.tensor.matmul(
                        ops[:], r(va[j][:, h * 65:(h + 1) * 65]), r(exps[j][:]),
                        start=(j == 0), stop=(j == KC - 1),
                    )
                pend = (h, ops)
            emit_norm(*pend)

            # ---- stage 3: y^T = Wproj @ out^T + b ---------------------
            for m in range(8):
                yw = wstat_pool.tile([128, C], F32R, tag="wstat")
                for k in range(KC):
                    nc.sync.dma_start(yw[:, k * 128:(k + 1) * 128], wp_d[k, m])
                ps = ps_pool.tile([128, TQ], F32, tag="ps")
                for k in range(KC):
                    nc.tensor.matmul(
                        ps[:], r(yw[:, k * 128:(k + 1) * 128]), r(ot[k][:]),
                        start=(k == 0), stop=(k == KC - 1),
                    )
                ysb = y_pool.tile([128, TQ], F32, tag="ysb")
                nc.scalar.activation(ysb[:], ps[:], Ident, bias=bias_sb[m][:])
                nc.sync.dma_start(yt_d[m], ysb[:])

    nc.compile()
    return nc


def _get_program():
    global _PROG
    if _PROG is None:
        _PROG = _build_program()
    return _PROG


def _prep_inputs(x, Wqkv, Wproj, bproj):
    """Host-side shard prep: per-core input maps (all fp32 contiguous)."""
    x = np.asarray(x, dtype=np.float32)
    Wqkv = np.asarray(Wqkv, dtype=np.float32)
    Wproj = np.asarray(Wproj, dtype=np.float32)
    bproj = np.asarray(bproj, dtype=np.float32)

    def tiles(wT):  # [C, C] (c, o) -> [8, 8, 128, 128] (c_chunk, o_chunk)
        return np.ascontiguousarray(
            wT.reshape(KC, 128, 8, 128).transpose(0, 2, 1, 3)
        )

    wq = tiles(Wqkv[0:C].T)
    wk = tiles(Wqkv[C:2 * C].T)
    wv = np.ascontiguousarray(Wqkv[2 * C:3 * C].T.reshape(KC, 128, C))
    wp = tiles(Wproj.T)
    bias = np.ascontiguousarray(bproj.reshape(8, 128, 1))

    in_maps = []
    for i in range(8):
        b, q = divmod(i, 2)
        xb = x[b]
        if q == 0:
            rot = xb
        else:
            rot = np.concatenate([xb[TQ:], xb[:TQ]], axis=0)
        xt = np.ascontiguousarray(rot.T.reshape(KC, 128, T))
        in_maps.append(
            {
                "xt": xt, "wq": wq, "wk": wk, "wv": wv, "wp": wp,
                "bias": bias, "ones": np.ones((128, 128), dtype=np.float32),
            }
        )
    return in_maps


def _assemble(results, x_dtype):
    out = np.empty((B, T, C), dtype=np.float32)
    for i in range(8):
        b, q = divmod(i, 2)
        yt = results[i]["yt"]  # [8, 128, TQ] = y^T chunked over o
        out[b, q * TQ:(q + 1) * TQ, :] = yt.reshape(C, TQ).T
    return out.astype(x_dtype, copy=False)


def run(inputs, trace=False, **spmd_kwargs):
    """Shared entry for kernel() and test harnesses (trace for profiling)."""
    from concourse.bass_utils import run_bass_kernel_spmd

    nc = _get_program()
    in_maps = _prep_inputs(**inputs)
    res = run_bass_kernel_spmd(
        nc, in_maps, list(range(8)), trace=trace, **spmd_kwargs
    )
    out = _assemble(res.results, np.asarray(inputs["x"]).dtype)
    return out, res


def kernel(x, Wqkv, Wproj, bproj):
    out, _ = run(dict(x=x, Wqkv=Wqkv, Wproj=Wproj, bproj=bproj))
    return out


# revision 11
# speedup vs baseline: 1.0328x; 1.0328x over previous
"""Trainium2 Bass kernel for nn_Attention (B=4, T=1024, C=1024, 16 heads).

Sharding: 8 cores = (batch b, query-half q). Core i handles queries
t in [q*512, q*512+512) of batch b = i//2, computing K/V for the whole
batch locally (no collectives). Host gather is a pure concatenation.

Everything on-chip is laid out to avoid transposes:
  - host passes x^T (tokens rotated so the query half is first; key
    order is a permutation, which softmax-attention is invariant to)
  - Q^T, K^T come out of the projection in [o, t] layout directly
  - V is produced in [t, o] layout with a ones-column appended per head
    so the PV matmul also yields the softmax denominator Z for free
  - softmax runs on S^T = (QK^T)^T, normalization multiplies by a
    PE-broadcast reciprocal row, and the output projection consumes
    out^T [c2, t] directly, yielding y^T which the host transposes back.

Schedule: V first (streaming behind the interleaved xt/wv DMAs), then
per head-pair group: K^T chunk, Q^T chunk, two heads of SDPA — so the
ScalarE exp work overlaps the projection matmuls and TensorE never
idles long enough to drop out of the 2.4 GHz HAM state.

All matmuls are float32r (TF32-like 4-byte fast path, 1 cycle/row for
N=512 moving operands) with fp32 PSUM accumulation.
"""

import numpy as np

B, T, C = 4, 1024, 1024
NH, HD = 16, 64
TQ = T // 2
KC = C // 128  # 8 contraction chunks
SCALE = 1.0 / float(np.sqrt(HD))

_PROG = None


def _build_program():
    import concourse.bacc as bacc
    import concourse.mybir as mybir
    import concourse.tile as tile

    F32 = mybir.dt.float32
    F32R = mybir.dt.float32r
    Exp = mybir.ActivationFunctionType.Exp
    Ident = mybir.ActivationFunctionType.Identity

    def r(ap):
        return ap.bitcast(F32R)

    nc = bacc.Bacc()
    xt_d = nc.declare_dram_parameter("xt", [KC, 128, T], F32R, isOutput=False)
    wq_d = nc.declare_dram_parameter("wq", [KC, 128, C], F32R, isOutput=False)
    wk_d = nc.declare_dram_parameter("wk", [KC, 128, C], F32R, isOutput=False)
    wv_d = nc.declare_dram_parameter("wv", [KC, 128, C], F32R, isOutput=False)
    wp_d = nc.declare_dram_parameter("wp", [KC, 128, C], F32R, isOutput=False)
    bias_d = nc.declare_dram_parameter("bias", [8, 128, 1], F32, isOutput=False)
    ones_d = nc.declare_dram_parameter("ones", [128, 128], F32R, isOutput=False)
    yt_d = nc.declare_dram_parameter("yt", [8, 128, TQ], F32, isOutput=True)

    from contextlib import ExitStack

    with ExitStack() as ctx:
        tc = ctx.enter_context(tile.TileContext(nc))
        ctx.enter_context(
            nc.allow_low_precision(
                "float32r matmul inputs (TF32-like) are intentional"
            )
        )
        pool = lambda name, bufs, **kw: ctx.enter_context(  # noqa: E731
            tc.tile_pool(name=name, bufs=bufs, **kw)
        )
        xt_pool = pool("xt", KC)
        wstat_pool = pool("wstat", 3)
        wmov_pool = pool("wmov", KC)
        kt_pool = pool("kt", KC)
        qt_pool = pool("qt", KC)
        v_pool = pool("vaug", KC)
        exp_pool = pool("exp", 8)
        ot_pool = pool("ot", KC)
        small_pool = pool("small", 1)
        y_pool = pool("ysb", 2)
        osb_pool = pool("osb", 2)
        osb2_pool = pool("osb2", 2)
        r_pool = pool("rsb", 2)
        bias_pool = pool("bias", 8)
        ps_proj = pool("psproj", 2, space="PSUM")
        ps_s = pool("pss", 3, space="PSUM")
        ps_ops = pool("psops", 2, space="PSUM")
        ps_rb = pool("psrb", 1, space="PSUM")
        if True:
            # ---- stage 0: streamed loads (xt/wv interleaved) ----------
            xt, wv_sb = [], []
            for k in range(KC):
                t_ = xt_pool.tile([128, T], F32R, tag="xt", name=f"xt{k}")
                nc.sync.dma_start(t_[:], xt_d[k])
                xt.append(t_)
                wvt = wmov_pool.tile([128, C], F32R, tag="wmov", name=f"wv{k}")
                nc.sync.dma_start(wvt[:], wv_d[k])
                wv_sb.append(wvt)

            ones65 = small_pool.tile([65, 128], F32R, tag="ones")
            nc.sync.dma_start(ones65[64:65, :], ones_d[0:1, :])

            bias_sb = []
            for m in range(8):
                bt = bias_pool.tile([128, 1], F32, tag="bias", name=f"bias{m}")
                nc.sync.dma_start(bt[:], bias_d[m])
                bias_sb.append(bt)

            va = []
            for m in range(KC):
                vt = v_pool.tile([128, NH * 65], F32R, tag="vaug", name=f"va{m}")
                view = vt[:].rearrange("p (h e) -> p h e", e=65)
                nc.sync.dma_start(view[:, :, 64:65], ones_d[:, 0:NH])
                va.append(vt)

            # ---- stage 1: V = x @ Wv^T  ([t,o] + ones columns) --------
            # k-inner accumulation consumes xt[k]/wv[k] in DMA arrival
            # order, so TensorE starts ~1 MB into the load stream.
            for m in range(KC):
                view = va[m][:].rearrange("p (h e) -> p h e", e=65)
                for n in range(2):
                    ps = ps_proj.tile([128, 512], F32, tag="ps", name=f"v{m}{n}")
                    for k in range(KC):
                        nc.tensor.matmul(
                            ps[:], r(xt[k][:, m * 128:(m + 1) * 128]),
                            r(wv_sb[k][:, n * 512:(n + 1) * 512]),
                            start=(k == 0), stop=(k == KC - 1),
                        )
                    src = ps[:].rearrange("p (h d) -> p h d", d=64)
                    nc.vector.tensor_copy(view[:, n * 8:(n + 1) * 8, 0:64], src)

            # ---- stages 2+3 interleaved: per head-pair group ----------
            # K^T chunk, Q^T chunk, then SDPA for its two heads.
            ot = []
            for k in range(KC):
                o_ = ot_pool.tile([128, TQ], F32R, tag="ot", name=f"ot{k}")
                ot.append(o_)

            def emit_norm(h, ops):
                okc, half = divmod(h, 2)
                po = half * 64
                rt = r_pool.tile([65, TQ], F32R, tag="rsb", name=f"r{h}")
                nc.vector.reciprocal(rt[64:65, :], ops[64:65, :])
                rb = ps_rb.tile([128, TQ], F32, tag="ps", name=f"rb{h}")
                nc.tensor.matmul(
                    rb[:], r(ones65[64:65, :]), r(rt[64:65, :]),
                    start=True, stop=True,
                )
                ob = osb_pool.tile([64, TQ], F32, tag="osb", name=f"ob{h}")
                nc.vector.tensor_copy(ob[:], ops[0:64, :])
                ob2 = osb2_pool.tile([64, TQ], F32R, tag="osb2", name=f"ob2_{h}")
                nc.vector.tensor_mul(ob2[:], ob[:], rb[0:64, :])
                nc.sync.dma_start(ot[okc][po:po + 64, :], ob2[:])

            qt, kt = [], []
            pend = None
            for kc in range(KC):
                # K^T chunk kc
                kw = wstat_pool.tile([128, C], F32R, tag="wstat", name=f"kw{kc}")
                nc.sync.dma_start(kw[:], wk_d[kc])
                k_ = kt_pool.tile([128, T], F32R, tag="kt", name=f"kt{kc}")
                for n in range(2):
                    ps = ps_proj.tile([128, 512], F32, tag="ps", name=f"k{kc}{n}")
                    for k in range(KC):
                        nc.tensor.matmul(
                            ps[:], r(kw[:, k * 128:(k + 1) * 128]),
                            r(xt[k][:, n * 512:(n + 1) * 512]),
                            start=(k == 0), stop=(k == KC - 1),
                        )
                    nc.vector.tensor_copy(k_[:, n * 512:(n + 1) * 512], ps[:])
                kt.append(k_)

                # Q^T chunk kc
                qw = wstat_pool.tile([128, C], F32R, tag="wstat", name=f"qw{kc}")
                nc.sync.dma_start(qw[:], wq_d[kc])
                ps = ps_proj.tile([128, TQ], F32, tag="ps", name=f"q{kc}")
                for k in range(KC):
                    nc.tensor.matmul(
                        ps[:], r(qw[:, k * 128:(k + 1) * 128]), r(xt[k][:, 0:TQ]),
                        start=(k == 0), stop=(k == KC - 1),
                    )
                q_ = qt_pool.tile([128, TQ], F32R, tag="qt", name=f"qt{kc}")
                nc.vector.tensor_copy(q_[:], ps[:])
                qt.append(q_)

                # SDPA for heads 2*kc, 2*kc+1
                for h in (2 * kc, 2 * kc + 1):
                    po = (h % 2) * 64
                    exps = []
                    for j in range(KC):
                        sps = ps_s.tile([128, TQ], F32, tag="ps", name=f"s{h}{j}")
                        nc.tensor.matmul(
                            sps[:],
                            r(kt[kc][po:po + 64, j * 128:(j + 1) * 128]),
                            r(qt[kc][po:po + 64, :]),
                            start=True, stop=True,
                        )
                        e = exp_pool.tile([128, TQ], F32R, tag="exp", name=f"e{h}{j}")
                        nc.scalar.activation(e[:], sps[:], Exp, scale=SCALE)
                        exps.append(e)
                    if pend is not None:
                        emit_norm(*pend)
                    ops = ps_ops.tile([65, TQ], F32, tag="ps", name=f"o{h}")
                    for j in range(KC):
                        nc.tensor.matmul(
                            ops[:], r(va[j][:, h * 65:(h + 1) * 65]), r(exps[j][:]),
                            start=(j == 0), stop=(j == KC - 1),
                        )
                    pend = (h, ops)
            emit_norm(*pend)

            # ---- stage 4: y^T = Wproj @ out^T + b ---------------------
            for m in range(8):
                yw = wstat_pool.tile([128, C], F32R, tag="wstat", name=f"yw{m}")
                nc.sync.dma_start(yw[:], wp_d[m])
                ps = ps_proj.tile([128, TQ], F32, tag="ps", name=f"y{m}")
                for k in range(KC):
                    nc.tensor.matmul(
                        ps[:], r(yw[:, k * 128:(k + 1) * 128]), r(ot[k][:]),
                        start=(k == 0), stop=(k == KC - 1),
                    )
                ysb = y_pool.tile([128, TQ], F32, tag="ysb", name=f"ysb{m}")
                nc.scalar.activation(ysb[:], ps[:], Ident, bias=bias_sb[m][:])
                nc.sync.dma_start(yt_d[m], ysb[:])

    nc.compile()
    return nc


def _get_program():
    global _PROG
    if _PROG is None:
        _PROG = _build_program()
    return _PROG


def _prep_inputs(x, Wqkv, Wproj, bproj):
    """Host-side shard prep: per-core input maps (all fp32 contiguous)."""
    x = np.asarray(x, dtype=np.float32)
    Wqkv = np.asarray(Wqkv, dtype=np.float32)
    Wproj = np.asarray(Wproj, dtype=np.float32)
    bproj = np.asarray(bproj, dtype=np.float32)

    def cols(wT):
        # [C, C] (c, o) -> [8, 128, 1024]: per o-chunk column, laid out
        # so one contiguous 512KB DMA fills the SBUF stationary tile
        # [128p, k*128+d] = wT[k*128+p, ko*128+d]
        return np.ascontiguousarray(
            wT.reshape(KC, 128, 8, 128).transpose(2, 1, 0, 3).reshape(8, 128, C)
        )

    wq = cols(Wqkv[0:C].T)
    wk = cols(Wqkv[C:2 * C].T)
    wv = np.ascontiguousarray(Wqkv[2 * C:3 * C].T.reshape(KC, 128, C))
    wp = cols(Wproj.T)
    bias = np.ascontiguousarray(bproj.reshape(8, 128, 1))
    ones = np.ones((128, 128), dtype=np.float32)

    in_maps = []
    for i in range(8):
        b, q = divmod(i, 2)
        xb = x[b]
        if q == 0:
            rot = xb
        else:
            rot = np.concatenate([xb[TQ:], xb[:TQ]], axis=0)
        xt = np.ascontiguousarray(rot.T.reshape(KC, 128, T))
        in_maps.append(
            {
                "xt": xt, "wq": wq, "wk": wk, "wv": wv, "wp": wp,
                "bias": bias, "ones": ones,
            }
        )
    return in_maps


def _assemble(results, x_dtype):
    out = np.empty((B, T, C), dtype=np.float32)
    for i in range(8):
        b, q = divmod(i, 2)
        yt = results[i]["yt"]  # [8, 128, TQ] = y^T chunked over o
        out[b, q * TQ:(q + 1) * TQ, :] = yt.reshape(C, TQ).T
    return out.astype(x_dtype, copy=False)


def run(inputs, trace=False, **spmd_kwargs):
    """Shared entry for kernel() and test harnesses (trace for profiling)."""
    from concourse.bass_utils import run_bass_kernel_spmd

    nc = _get_program()
    in_maps = _prep_inputs(**inputs)
    res = run_bass_kernel_spmd(
        nc, in_maps, list(range(8)), trace=trace, **spmd_kwargs
    )
    out = _assemble(res.results, np.asarray(inputs["x"]).dtype)
    return out, res


def kernel(x, Wqkv, Wproj, bproj):
    out, _ = run(dict(x=x, Wqkv=Wqkv, Wproj=Wproj, bproj=bproj))
    return out


# revision 23
# speedup vs baseline: 1.4815x; 1.4344x over previous
"""Trainium2 Bass kernel for nn_Attention (B=4, T=1024, C=1024, 16 heads).

Sharding: 8 cores = (batch b, query-half q). Core i handles queries
t in [q*512, q*512+512) of batch b = i//2, computing K/V for the whole
batch locally (no collectives). Host gather is a pure concatenation.

Everything on-chip is laid out to avoid transposes:
  - host passes x^T (tokens rotated so the query half is first; key
    order is a permutation, which softmax-attention is invariant to)
  - Q^T, K^T come out of the projection in [o, t] layout directly
  - V is produced in [t, o] layout with a ones-column appended per head
    so the PV matmul also yields the softmax denominator Z for free
  - softmax runs on S^T = (QK^T)^T, normalization multiplies by a
    PE-broadcast reciprocal row, and the output projection consumes
    out^T [c2, t] directly, yielding y^T which the host transposes back.

Schedule: V first (streaming behind the interleaved xt/wv DMAs), then
per head-pair group: K^T chunk, Q^T chunk, two heads of SDPA — so the
ScalarE exp work overlaps the projection matmuls and TensorE never
idles long enough to drop out of the 2.4 GHz HAM state.

All matmuls are float32r (TF32-like 4-byte fast path, 1 cycle/row for
N=512 moving operands) with fp32 PSUM accumulation.
"""

import numpy as np

B, T, C = 4, 1024, 1024
NH, HD = 16, 64
TQ = T // 2
KC = C // 128  # 8 contraction chunks
SCALE = 1.0 / float(np.sqrt(HD))

_PROG = None


def _build_program():
    import concourse.bacc as bacc
    import concourse.mybir as mybir
    import concourse.tile as tile

    F32 = mybir.dt.float32
    F32R = mybir.dt.float32r
    Exp = mybir.ActivationFunctionType.Exp
    Ident = mybir.ActivationFunctionType.Identity

    def r(ap):
        return ap.bitcast(F32R)

    nc = bacc.Bacc()
    xt_d = nc.declare_dram_parameter("xt", [KC, 128, T], F32R, isOutput=False)
    wq_d = nc.declare_dram_parameter("wq", [KC, 128, C], F32R, isOutput=False)
    wk_d = nc.declare_dram_parameter("wk", [KC, 128, C], F32R, isOutput=False)
    wv_d = nc.declare_dram_parameter("wv", [KC, 128, C], F32R, isOutput=False)
    wp_d = nc.declare_dram_parameter("wp", [KC, 128, C], F32R, isOutput=False)
    bias_d = nc.declare_dram_parameter("bias", [8, 128, 1], F32, isOutput=False)
    ones_d = nc.declare_dram_parameter("ones", [128, 128], F32R, isOutput=False)
    yt_d = nc.declare_dram_parameter("yt", [8, 128, TQ], F32, isOutput=True)

    from contextlib import ExitStack

    with ExitStack() as ctx:
        tc = ctx.enter_context(tile.TileContext(nc))
        ctx.enter_context(
            nc.allow_low_precision(
                "float32r matmul inputs (TF32-like) are intentional"
            )
        )
        pool = lambda name, bufs, **kw: ctx.enter_context(  # noqa: E731
            tc.tile_pool(name=name, bufs=bufs, **kw)
        )
        xt_pool = pool("xt", KC)
        wstat_pool = pool("wstat", 3)
        # wv tiles live only during the V stage; exp tiles (same size)
        # reuse the same 8 slots afterwards via the shared tag.
        wmov_pool = pool("wmov", KC)
        kt_pool = pool("kt", KC)
        qt_pool = pool("qt", KC)
        v_pool = pool("vaug", KC)
        ot_pool = pool("ot", KC)
        small_pool = pool("small", 1)
        y_pool = pool("ysb", 2)
        osb_pool = pool("osb", 2)
        osb2_pool = pool("osb2", 2)
        r_pool = pool("rsb", 2)
        rbx_pool = pool("rbx", 2)
        bias_pool = pool("bias", 8)
        ps_proj = pool("psproj", 2, space="PSUM")
        ps_s = pool("pss", 2, space="PSUM")  # [128,1024] pair tiles, 2 banks each
        ps_ops = pool("psops", 2, space="PSUM")
        if True:
            # ---- stage 0: streamed loads (xt/wv interleaved) ----------
            xt, wv_sb = [], []
            for k in range(KC):
                t_ = xt_pool.tile([128, T], F32R, tag="xt", name=f"xt{k}")
                nc.sync.dma_start(t_[:], xt_d[k])
                xt.append(t_)
                wvt = wmov_pool.tile([128, C], F32R, tag="wmov", name=f"wv{k}")
                nc.sync.dma_start(wvt[:], wv_d[k])
                wv_sb.append(wvt)

            bias_sb = []
            for m in range(8):
                bt = bias_pool.tile([128, 1], F32, tag="bias", name=f"bias{m}")
                nc.sync.dma_start(bt[:], bias_d[m])
                bias_sb.append(bt)

            va = []
            for m in range(KC):
                vt = v_pool.tile([128, NH * 65], F32R, tag="vaug", name=f"va{m}")
                view = vt[:].rearrange("p (h e) -> p h e", e=65)
                nc.sync.dma_start(view[:, :, 64:65], ones_d[:, 0:NH])
                va.append(vt)

            # ---- stage 1: V = x @ Wv^T  ([t,o] + ones columns) --------
            # k-inner accumulation consumes xt[k]/wv[k] in DMA arrival
            # order, so TensorE starts ~1 MB into the load stream.
            for m in range(KC):
                view = va[m][:].rearrange("p (h e) -> p h e", e=65)
                for n in range(2):
                    ps = ps_proj.tile([128, 512], F32, tag="ps", name=f"v{m}{n}")
                    for k in range(KC):
                        nc.tensor.matmul(
                            ps[:], r(xt[k][:, m * 128:(m + 1) * 128]),
                            r(wv_sb[k][:, n * 512:(n + 1) * 512]),
                            start=(k == 0), stop=(k == KC - 1),
                        )
                    src = ps[:].rearrange("p (h d) -> p h d", d=64)
                    nc.vector.tensor_copy(view[:, n * 8:(n + 1) * 8, 0:64], src)

            # ---- stages 2+3 interleaved: per head-pair group ----------
            # K^T chunk, Q^T chunk, then SDPA for its two heads.
            ot = []
            for k in range(KC):
                o_ = ot_pool.tile([128, TQ], F32R, tag="ot", name=f"ot{k}")
                ot.append(o_)

            def emit_norm(h, ops):
                okc, half = divmod(h, 2)
                po = half * 64
                # 1/Z on DVE (fast approx, 18 bits), broadcast across
                # partitions on idle GpSimd, multiply on DVE. No PE work.
                # 1/Z on partition 64 (aligned with the PSUM Z row), then
                # DMA-shift the row to partition 0 for GpSimd's broadcast
                # (which reads physical partition 0).
                rt = r_pool.tile([65, TQ], F32, tag="rsb", name=f"r{h}")
                nc.vector.reciprocal(rt[64:65, :], ops[64:65, :])
                rt0 = r_pool.tile([1, TQ], F32, tag="rsb2", name=f"r0{h}")
                nc.sync.dma_start(rt0[0:1, :], rt[64:65, :])
                rbx = rbx_pool.tile([64, TQ], F32, tag="rbx", name=f"rbx{h}")
                nc.gpsimd.partition_broadcast(rbx[:], rt0[0:1, :])
                ob = osb_pool.tile([64, TQ], F32, tag="osb", name=f"ob{h}")
                nc.vector.tensor_copy(ob[:], ops[0:64, :])
                ob2 = osb2_pool.tile([64, TQ], F32R, tag="osb2", name=f"ob2_{h}")
                nc.vector.tensor_mul(ob2[:], ob[:], rbx[:])
                nc.sync.dma_start(ot[okc][po:po + 64, :], ob2[:])

            qt, kt = [], []
            for kc in range(KC):
                # K^T chunk kc
                kw = wstat_pool.tile([128, C], F32R, tag="wstat", name=f"kw{kc}")
                nc.sync.dma_start(kw[:], wk_d[kc])
                k_ = kt_pool.tile([128, T], F32R, tag="kt", name=f"kt{kc}")
                for n in range(2):
                    ps = ps_proj.tile([128, 512], F32, tag="ps", name=f"k{kc}{n}")
                    for k in range(KC):
                        nc.tensor.matmul(
                            ps[:], r(kw[:, k * 128:(k + 1) * 128]),
                            r(xt[k][:, n * 512:(n + 1) * 512]),
                            start=(k == 0), stop=(k == KC - 1),
                        )
                    nc.vector.tensor_copy(k_[:, n * 512:(n + 1) * 512], ps[:])
                kt.append(k_)

                # Q^T chunk kc
                qw = wstat_pool.tile([128, C], F32R, tag="wstat", name=f"qw{kc}")
                nc.sync.dma_start(qw[:], wq_d[kc])
                ps = ps_proj.tile([128, TQ], F32, tag="ps", name=f"q{kc}")
                for k in range(KC):
                    nc.tensor.matmul(
                        ps[:], r(qw[:, k * 128:(k + 1) * 128]), r(xt[k][:, 0:TQ]),
                        start=(k == 0), stop=(k == KC - 1),
                    )
                q_ = qt_pool.tile([128, TQ], F32R, tag="qt", name=f"qt{kc}")
                nc.vector.tensor_copy(q_[:], ps[:])
                qt.append(q_)

                # SDPA for the head pair (h0, h1) = (2kc, 2kc+1).
                # Both heads' S^T chunk j share one 2-bank PSUM tile so a
                # single [128,1024] exp covers them (halves ACT overhead).
                h0, h1 = 2 * kc, 2 * kc + 1
                exps = []
                for j in range(KC):
                    sps = ps_s.tile([128, 2 * TQ], F32, tag="ps", name=f"s{kc}{j}")
                    nc.tensor.matmul(
                        sps[:, 0:TQ],
                        r(kt[kc][0:64, j * 128:(j + 1) * 128]),
                        r(qt[kc][0:64, :]),
                        start=True, stop=True,
                    )
                    nc.tensor.matmul(
                        sps[:, TQ:2 * TQ],
                        r(kt[kc][64:128, j * 128:(j + 1) * 128]),
                        r(qt[kc][64:128, :]),
                        start=True, stop=True,
                    )
                    e = wmov_pool.tile([128, 2 * TQ], F32R, tag="wmov",
                                       name=f"e{kc}{j}")
                    nc.scalar.activation(e[:], sps[:], Exp, scale=SCALE)
                    exps.append(e)
                for h, lo in ((h0, 0), (h1, TQ)):
                    ops = ps_ops.tile([65, TQ], F32, tag="ps", name=f"o{h}")
                    for j in range(KC):
                        nc.tensor.matmul(
                            ops[:], r(va[j][:, h * 65:(h + 1) * 65]),
                            r(exps[j][:, lo:lo + TQ]),
                            start=(j == 0), stop=(j == KC - 1),
                        )
                    emit_norm(h, ops)

            # ---- stage 4: y^T = Wproj @ out^T + b ---------------------
            for m in range(8):
                yw = wstat_pool.tile([128, C], F32R, tag="wstat", name=f"yw{m}")
                nc.sync.dma_start(yw[:], wp_d[m])
                ps = ps_proj.tile([128, TQ], F32, tag="ps", name=f"y{m}")
                for k in range(KC):
                    nc.tensor.matmul(
                        ps[:], r(yw[:, k * 128:(k + 1) * 128]), r(ot[k][:]),
                        start=(k == 0), stop=(k == KC - 1),
                    )
                ysb = y_pool.tile([128, TQ], F32, tag="ysb", name=f"ysb{m}")
                nc.scalar.activation(ysb[:], ps[:], Ident, bias=bias_sb[m][:])
                nc.sync.dma_start(yt_d[m], ysb[:])

    nc.compile()
    return nc


def _get_program():
    global _PROG
    if _PROG is None:
        _PROG = _build_program()
    return _PROG


def _prep_inputs(x, Wqkv, Wproj, bproj):
    """Host-side shard prep: per-core input maps (all fp32 contiguous)."""
    x = np.asarray(x, dtype=np.float32)
    Wqkv = np.asarray(Wqkv, dtype=np.float32)
    Wproj = np.asarray(Wproj, dtype=np.float32)
    bproj = np.asarray(bproj, dtype=np.float32)

    def cols(wT):
        # [C, C] (c, o) -> [8, 128, 1024]: per o-chunk column, laid out
        # so one contiguous 512KB DMA fills the SBUF stationary tile
        # [128p, k*128+d] = wT[k*128+p, ko*128+d]
        return np.ascontiguousarray(
            wT.reshape(KC, 128, 8, 128).transpose(2, 1, 0, 3).reshape(8, 128, C)
        )

    wq = cols(Wqkv[0:C].T)
    wk = cols(Wqkv[C:2 * C].T)
    wv = np.ascontiguousarray(Wqkv[2 * C:3 * C].T.reshape(KC, 128, C))
    wp = cols(Wproj.T)
    bias = np.ascontiguousarray(bproj.reshape(8, 128, 1))
    ones = np.ones((128, 128), dtype=np.float32)

    in_maps = []
    for i in range(8):
        b, q = divmod(i, 2)
        xb = x[b]
        if q == 0:
            rot = xb
        else:
            rot = np.concatenate([xb[TQ:], xb[:TQ]], axis=0)
        xt = np.ascontiguousarray(rot.T.reshape(KC, 128, T))
        in_maps.append(
            {
                "xt": xt, "wq": wq, "wk": wk, "wv": wv, "wp": wp,
                "bias": bias, "ones": ones,
            }
        )
    return in_maps


def _assemble(results, x_dtype):
    out = np.empty((B, T, C), dtype=np.float32)
    for i in range(8):
        b, q = divmod(i, 2)
        yt = results[i]["yt"]  # [8, 128, TQ] = y^T chunked over o
        out[b, q * TQ:(q + 1) * TQ, :] = yt.reshape(C, TQ).T
    return out.astype(x_dtype, copy=False)


def run(inputs, trace=False, **spmd_kwargs):
    """Shared entry for kernel() and test harnesses (trace for profiling)."""
    from concourse.bass_utils import run_bass_kernel_spmd

    nc = _get_program()
    in_maps = _prep_inputs(**inputs)
    res = run_bass_kernel_spmd(
        nc, in_maps, list(range(8)), trace=trace, **spmd_kwargs
    )
    out = _assemble(res.results, np.asarray(inputs["x"]).dtype)
    return out, res


def kernel(x, Wqkv, Wproj, bproj):
    out, _ = run(dict(x=x, Wqkv=Wqkv, Wproj=Wproj, bproj=bproj))
    return out


# revision 25
# speedup vs baseline: 1.9443x; 1.3124x over previous
"""Trainium2 Bass kernel for nn_Attention (B=4, T=1024, C=1024, 16 heads).

Sharding: 8 cores = (batch b, query-half q). Core i handles queries
t in [q*512, q*512+512) of batch b = i//2, computing K/V for the whole
batch locally (no collectives). Host gather is a pure concatenation.

Everything on-chip is laid out to avoid transposes:
  - host passes x^T (tokens rotated so the query half is first; key
    order is a permutation, which softmax-attention is invariant to)
  - Q^T, K^T come out of the projection in [o, t] layout directly
  - V is produced in [t, o] layout with a ones-column appended per head
    so the PV matmul also yields the softmax denominator Z for free
  - softmax runs on S^T = (QK^T)^T, normalization multiplies by a
    PE-broadcast reciprocal row, and the output projection consumes
    out^T [c2, t] directly, yielding y^T which the host transposes back.

Schedule: V first (streaming behind the interleaved xt/wv DMAs), then
per head-pair group: K^T chunk, Q^T chunk, two heads of SDPA — so the
ScalarE exp work overlaps the projection matmuls and TensorE never
idles long enough to drop out of the 2.4 GHz HAM state.

All matmuls are float32r (TF32-like 4-byte fast path, 1 cycle/row for
N=512 moving operands) with fp32 PSUM accumulation.
"""

import numpy as np

B, T, C = 4, 1024, 1024
NH, HD = 16, 64
TQ = T // 2
KC = C // 128  # 8 contraction chunks
SCALE = 1.0 / float(np.sqrt(HD))

_PROG = None


def _build_program():
    import concourse.bacc as bacc
    import concourse.mybir as mybir
    import concourse.tile as tile

    F32 = mybir.dt.float32
    F32R = mybir.dt.float32r
    Exp = mybir.ActivationFunctionType.Exp
    Ident = mybir.ActivationFunctionType.Identity
    Copy = mybir.ActivationFunctionType.Copy

    def r(ap):
        return ap.bitcast(F32R)

    nc = bacc.Bacc()
    xt_d = nc.declare_dram_parameter("xt", [KC, 128, T], F32R, isOutput=False)
    wq_d = nc.declare_dram_parameter("wq", [KC, 128, C], F32R, isOutput=False)
    wk_d = nc.declare_dram_parameter("wk", [KC, 128, C], F32R, isOutput=False)
    wv_d = nc.declare_dram_parameter("wv", [KC, 128, C], F32R, isOutput=False)
    wp_d = nc.declare_dram_parameter("wp", [KC, 128, C], F32R, isOutput=False)
    bias_d = nc.declare_dram_parameter("bias", [8, 128, 1], F32, isOutput=False)
    ones_d = nc.declare_dram_parameter("ones", [128, 128], F32R, isOutput=False)
    yt_d = nc.declare_dram_parameter("yt", [8, 128, TQ], F32, isOutput=True)

    from contextlib import ExitStack

    with ExitStack() as ctx:
        tc = ctx.enter_context(tile.TileContext(nc))
        ctx.enter_context(
            nc.allow_low_precision(
                "float32r matmul inputs (TF32-like) are intentional"
            )
        )
        pool = lambda name, bufs, **kw: ctx.enter_context(  # noqa: E731
            tc.tile_pool(name=name, bufs=bufs, **kw)
        )
        xt_pool = pool("xt", KC)
        wstat_pool = pool("wstat", 3)
        # wv tiles live only during the V stage; exp tiles (same size)
        # reuse the same 8 slots afterwards via the shared tag.
        wmov_pool = pool("wmov", KC)
        kt_pool = pool("kt", KC)
        qt_pool = pool("qt", KC)
        v_pool = pool("vaug", KC)
        ot_pool = pool("ot", KC)
        small_pool = pool("small", 1)
        y_pool = pool("ysb", 2)
        osb_pool = pool("osb", 2)
        osb2_pool = pool("osb2", 2)
        r_pool = pool("rsb", 2)
        rbx_pool = pool("rbx", 2)
        bias_pool = pool("bias", 8)
        ps_proj = pool("psproj", 2, space="PSUM")
        ps_s = pool("pss", 2, space="PSUM")  # [128,1024] pair tiles, 2 banks each
        ps_ops = pool("psops", 2, space="PSUM")
        if True:
            # ---- stage 0: streamed loads (xt/wv interleaved) ----------
            xt, wv_sb = [], []
            for k in range(KC):
                t_ = xt_pool.tile([128, T], F32R, tag="xt", name=f"xt{k}")
                nc.sync.dma_start(t_[:], xt_d[k])
                xt.append(t_)
                wvt = wmov_pool.tile([128, C], F32R, tag="wmov", name=f"wv{k}")
                nc.sync.dma_start(wvt[:], wv_d[k])
                wv_sb.append(wvt)

            bias_sb = []
            for m in range(8):
                bt = bias_pool.tile([128, 1], F32, tag="bias", name=f"bias{m}")
                nc.sync.dma_start(bt[:], bias_d[m])
                bias_sb.append(bt)

            va = []
            for m in range(KC):
                vt = v_pool.tile([128, NH * 65], F32R, tag="vaug", name=f"va{m}")
                view = vt[:].rearrange("p (h e) -> p h e", e=65)
                nc.sync.dma_start(view[:, :, 64:65], ones_d[:, 0:NH])
                va.append(vt)

            # ---- stage 1: V = x @ Wv^T  ([t,o] + ones columns) --------
            # k-inner accumulation consumes xt[k]/wv[k] in DMA arrival
            # order, so TensorE starts ~1 MB into the load stream.
            for m in range(KC):
                view = va[m][:].rearrange("p (h e) -> p h e", e=65)
                for n in range(2):
                    ps = ps_proj.tile([128, 512], F32, tag="ps", name=f"v{m}{n}")
                    for k in range(KC):
                        nc.tensor.matmul(
                            ps[:], r(xt[k][:, m * 128:(m + 1) * 128]),
                            r(wv_sb[k][:, n * 512:(n + 1) * 512]),
                            start=(k == 0), stop=(k == KC - 1),
                        )
                    src = ps[:].rearrange("p (h d) -> p h d", d=64)
                    nc.vector.tensor_copy(view[:, n * 8:(n + 1) * 8, 0:64], src)

            # ---- stages 2+3 interleaved: per head-pair group ----------
            # K^T chunk, Q^T chunk, then SDPA for its two heads.
            ot = []
            for k in range(KC):
                o_ = ot_pool.tile([128, TQ], F32R, tag="ot", name=f"ot{k}")
                ot.append(o_)

            def emit_norm(h, ops):
                okc, half = divmod(h, 2)
                po = half * 64
                # Z row (PSUM partition 64) -> SBUF, DMA-shift to partition
                # 0 (both the custom-DVE reciprocal and GpSimd's broadcast
                # only operate at physical partition 0), then fast 1/Z,
                # GpSimd broadcast, and one DVE multiply. No PE work.
                zc = r_pool.tile([65, TQ], F32, tag="rsb", name=f"z{h}")
                nc.vector.tensor_copy(zc[64:65, :], ops[64:65, :])
                z0 = r_pool.tile([1, TQ], F32, tag="rsb2", name=f"z0{h}")
                nc.sync.dma_start(z0[0:1, :], zc[64:65, :])
                rt0 = r_pool.tile([1, TQ], F32, tag="rsb3", name=f"r0{h}")
                nc.vector.reciprocal_approx_fast(rt0[0:1, :], z0[0:1, :])
                rbx = rbx_pool.tile([64, TQ], F32, tag="rbx", name=f"rbx{h}")
                nc.gpsimd.partition_broadcast(rbx[:], rt0[0:1, :])
                ob = osb_pool.tile([64, TQ], F32, tag="osb", name=f"ob{h}")
                nc.scalar.activation(ob[:], ops[0:64, :], Copy, bias=0.0)
                ob2 = osb2_pool.tile([64, TQ], F32R, tag="osb2", name=f"ob2_{h}")
                nc.vector.tensor_mul(ob2[:], ob[:], rbx[:])
                nc.sync.dma_start(ot[okc][po:po + 64, :], ob2[:])

            qt, kt = [], []
            for kc in range(KC):
                # K^T chunk kc
                kw = wstat_pool.tile([128, C], F32R, tag="wstat", name=f"kw{kc}")
                nc.sync.dma_start(kw[:], wk_d[kc])
                k_ = kt_pool.tile([128, T], F32R, tag="kt", name=f"kt{kc}")
                for n in range(2):
                    ps = ps_proj.tile([128, 512], F32, tag="ps", name=f"k{kc}{n}")
                    for k in range(KC):
                        nc.tensor.matmul(
                            ps[:], r(kw[:, k * 128:(k + 1) * 128]),
                            r(xt[k][:, n * 512:(n + 1) * 512]),
                            start=(k == 0), stop=(k == KC - 1),
                        )
                    nc.vector.tensor_copy(k_[:, n * 512:(n + 1) * 512], ps[:])
                kt.append(k_)

                # Q^T chunk kc
                qw = wstat_pool.tile([128, C], F32R, tag="wstat", name=f"qw{kc}")
                nc.sync.dma_start(qw[:], wq_d[kc])
                ps = ps_proj.tile([128, TQ], F32, tag="ps", name=f"q{kc}")
                for k in range(KC):
                    nc.tensor.matmul(
                        ps[:], r(qw[:, k * 128:(k + 1) * 128]), r(xt[k][:, 0:TQ]),
                        start=(k == 0), stop=(k == KC - 1),
                    )
                q_ = qt_pool.tile([128, TQ], F32R, tag="qt", name=f"qt{kc}")
                nc.vector.tensor_copy(q_[:], ps[:])
                qt.append(q_)

                # SDPA for the head pair (h0, h1) = (2kc, 2kc+1).
                # Both heads' S^T chunk j share one 2-bank PSUM tile so a
                # single [128,1024] exp covers them (halves ACT overhead).
                h0, h1 = 2 * kc, 2 * kc + 1
                exps = []
                for j in range(KC):
                    sps = ps_s.tile([128, 2 * TQ], F32, tag="ps", name=f"s{kc}{j}")
                    nc.tensor.matmul(
                        sps[:, 0:TQ],
                        r(kt[kc][0:64, j * 128:(j + 1) * 128]),
                        r(qt[kc][0:64, :]),
                        start=True, stop=True,
                    )
                    nc.tensor.matmul(
                        sps[:, TQ:2 * TQ],
                        r(kt[kc][64:128, j * 128:(j + 1) * 128]),
                        r(qt[kc][64:128, :]),
                        start=True, stop=True,
                    )
                    e = wmov_pool.tile([128, 2 * TQ], F32R, tag="wmov",
                                       name=f"e{kc}{j}")
                    nc.scalar.activation(e[:], sps[:], Exp, scale=SCALE)
                    exps.append(e)
                for h, lo in ((h0, 0), (h1, TQ)):
                    ops = ps_ops.tile([65, TQ], F32, tag="ps", name=f"o{h}")
                    for j in range(KC):
                        nc.tensor.matmul(
                            ops[:], r(va[j][:, h * 65:(h + 1) * 65]),
                            r(exps[j][:, lo:lo + TQ]),
                            start=(j == 0), stop=(j == KC - 1),
                        )
                    emit_norm(h, ops)

            # ---- stage 4: y^T = Wproj @ out^T + b ---------------------
            for m in range(8):
                yw = wstat_pool.tile([128, C], F32R, tag="wstat", name=f"yw{m}")
                nc.sync.dma_start(yw[:], wp_d[m])
                ps = ps_proj.tile([128, TQ], F32, tag="ps", name=f"y{m}")
                for k in range(KC):
                    nc.tensor.matmul(
                        ps[:], r(yw[:, k * 128:(k + 1) * 128]), r(ot[k][:]),
                        start=(k == 0), stop=(k == KC - 1),
                    )
                ysb = y_pool.tile([128, TQ], F32, tag="ysb", name=f"ysb{m}")
                nc.scalar.activation(ysb[:], ps[:], Ident, bias=bias_sb[m][:])
                nc.sync.dma_start(yt_d[m], ysb[:])

    nc.compile()
    return nc


def _get_program():
    global _PROG
    if _PROG is None:
        _PROG = _build_program()
    return _PROG


def _prep_inputs(x, Wqkv, Wproj, bproj):
    """Host-side shard prep: per-core input maps (all fp32 contiguous)."""
    x = np.asarray(x, dtype=np.float32)
    Wqkv = np.asarray(Wqkv, dtype=np.float32)
    Wproj = np.asarray(Wproj, dtype=np.float32)
    bproj = np.asarray(bproj, dtype=np.float32)

    def cols(wT):
        # [C, C] (c, o) -> [8, 128, 1024]: per o-chunk column, laid out
        # so one contiguous 512KB DMA fills the SBUF stationary tile
        # [128p, k*128+d] = wT[k*128+p, ko*128+d]
        return np.ascontiguousarray(
            wT.reshape(KC, 128, 8, 128).transpose(2, 1, 0, 3).reshape(8, 128, C)
        )

    wq = cols(Wqkv[0:C].T)
    wk = cols(Wqkv[C:2 * C].T)
    wv = np.ascontiguousarray(Wqkv[2 * C:3 * C].T.reshape(KC, 128, C))
    wp = cols(Wproj.T)
    bias = np.ascontiguousarray(bproj.reshape(8, 128, 1))
    ones = np.ones((128, 128), dtype=np.float32)

    in_maps = []
    for i in range(8):
        b, q = divmod(i, 2)
        xb = x[b]
        if q == 0:
            rot = xb
        else:
            rot = np.concatenate([xb[TQ:], xb[:TQ]], axis=0)
        xt = np.ascontiguousarray(rot.T.reshape(KC, 128, T))
        in_maps.append(
            {
                "xt": xt, "wq": wq, "wk": wk, "wv": wv, "wp": wp,
                "bias": bias, "ones": ones,
            }
        )
    return in_maps


def _assemble(results, x_dtype):
    out = np.empty((B, T, C), dtype=np.float32)
    for i in range(8):
        b, q = divmod(i, 2)
        yt = results[i]["yt"]  # [8, 128, TQ] = y^T chunked over o
        out[b, q * TQ:(q + 1) * TQ, :] = yt.reshape(C, TQ).T
    return out.astype(x_dtype, copy=False)


def run(inputs, trace=False, **spmd_kwargs):
    """Shared entry for kernel() and test harnesses (trace for profiling)."""
    from concourse.bass_utils import run_bass_kernel_spmd

    nc = _get_program()
    in_maps = _prep_inputs(**inputs)
    res = run_bass_kernel_spmd(
        nc, in_maps, list(range(8)), trace=trace, **spmd_kwargs
    )
    out = _assemble(res.results, np.asarray(inputs["x"]).dtype)
    return out, res


def kernel(x, Wqkv, Wproj, bproj):
    out, _ = run(dict(x=x, Wqkv=Wqkv, Wproj=Wproj, bproj=bproj))
    return out
